# revision 1
# baseline (speedup 1.0000x reference)
"""Trainium2 Bass kernel for nn_MemoryCore (retrieval KNN min-distance).

Problem: embedding [8192, 512], memory_bank [65536, 512] (fp32) ->
patch_scores [8192, 1] = min over the bank of euclidean distance.

Strategy (8 NeuronCores, SPMD):
  - Shard the memory bank (M axis) 8 ways; every core sees all queries.
  - Per core: psum[m, n] = (-2*bank_shard) @ emb.T via PE (float32r,
    1 cyc/row), fused running min over m-tiles on DVE via
    scalar_tensor_tensor: RM = min(psum + m_sq[m], RM).
  - Epilogue per 512-query block: PE-transpose RM, reduce_min over the
    free axis, sqrt(min + x_sq) on ACT -> per-core local min distances.
  - Host: elementwise min across the 8 cores.
"""
import numpy as np
import concourse.bacc as bacc
import concourse.mybir as mybir
import concourse.tile as tile
from concourse.bass_utils import run_bass_kernel_spmd
from concourse.masks import make_identity

N_CORES = 8
N, M, D = 8192, 65536, 512
MS = M // N_CORES       # 8192 bank rows per core
MSB = 1024              # bank chunk width (columns) per persistent tile
PSUM_BUFS = 6
BIG = 1e30
DT = mybir.dt.float32r  # TF32-like matmul: 4x faster than fp32, ~1e-4 rel err

_CACHE = {}


def _build_kernel():
    K = D // 128            # contraction chunks
    NB = N // 512           # query blocks (free axis)
    MT = MS // 128          # bank tiles (partitions)
    NMSB = MS // MSB
    mt_per_chunk = MSB // 128

    nc = bacc.Bacc("TRN2", target_bir_lowering=False, debug=False,
                   num_devices=N_CORES)

    embT_d = nc.dram_tensor("embT", [D, N], DT, kind="ExternalInput")
    bankT_d = nc.dram_tensor("bankT", [D, MS], DT, kind="ExternalInput")
    msq_d = nc.dram_tensor("msq", [128, MT], mybir.dt.float32, kind="ExternalInput")
    xsq_d = nc.dram_tensor("xsq", [128, N // 128], mybir.dt.float32, kind="ExternalInput")
    out_d = nc.dram_tensor("out", [128, N // 128], mybir.dt.float32, kind="ExternalOutput")

    with tile.TileContext(nc) as tc:
        with (
            tc.tile_pool(name="persist", bufs=1) as persist,
            tc.tile_pool(name="emb", bufs=2) as embp,
            tc.tile_pool(name="rmp", bufs=2) as rmp,
            tc.tile_pool(name="small", bufs=4) as small,
            tc.tile_pool(name="psum", bufs=PSUM_BUFS, space="PSUM") as psum,
            tc.tile_pool(name="psum_t", bufs=2, space="PSUM") as psum_t,
        ):
            msq = persist.tile([128, MT], mybir.dt.float32, tag="msq")
            nc.gpsimd.dma_start(msq[:], msq_d[:])
            xsq = persist.tile([128, N // 128], mybir.dt.float32, tag="xsq")
            nc.gpsimd.dma_start(xsq[:], xsq_d[:])
            out_s = persist.tile([128, N // 128], mybir.dt.float32, tag="outs")
            ident = persist.tile([128, 128], mybir.dt.float32, tag="ident")
            make_identity(nc, ident)

            def load_emb(nb):
                t = embp.tile([128, K, 512], DT, tag="embt")
                for k in range(K):
                    nc.gpsimd.dma_start(
                        t[:, k, :],
                        embT_d[k * 128:(k + 1) * 128, nb * 512:(nb + 1) * 512])
                return t

            emb_next = load_emb(0)

            bank_t = [[None] * NMSB for _ in range(K)]
            for j in range(NMSB):
                for k in range(K):
                    t = persist.tile([128, MSB], DT, tag=f"bank{k}_{j}")
                    nc.sync.dma_start(
                        t[:], bankT_d[k * 128:(k + 1) * 128, j * MSB:(j + 1) * MSB])
                    bank_t[k][j] = t

            for nb in range(NB):
                emb_t = emb_next
                if nb + 1 < NB:
                    emb_next = load_emb(nb + 1)
                rm = rmp.tile([128, 512], mybir.dt.float32, tag="rm")
                nc.vector.memset(rm[:], BIG)
                for mt in range(MT):
                    j, jj = mt // mt_per_chunk, mt % mt_per_chunk
                    ps = psum.tile([128, 512], mybir.dt.float32, tag="ps")
                    for k in range(K):
                        nc.tensor.matmul(
                            ps[:],
                            bank_t[k][j][:, jj * 128:(jj + 1) * 128],
                            emb_t[:, k, :],
                            start=(k == 0),
                            stop=(k == K - 1),
                        )
                    # RM = min(psum + m_sq[m], RM)  (one DVE op, reads PSUM)
                    nc.vector.scalar_tensor_tensor(
                        out=rm[:],
                        in0=ps[:],
                        scalar=msq[:, mt:mt + 1],
                        in1=rm[:],
                        op0=mybir.AluOpType.add,
                        op1=mybir.AluOpType.min,
                    )
                for q in range(4):  # cross-partition min per 128-query chunk
                    pt = psum_t.tile([128, 128], mybir.dt.float32, tag="pt")
                    nc.tensor.transpose(pt[:], rm[:, q * 128:(q + 1) * 128], ident[:])
                    mn = small.tile([128, 1], mybir.dt.float32, tag="mn")
                    nc.vector.tensor_reduce(
                        out=mn[:], in_=pt[:], axis=mybir.AxisListType.X,
                        op=mybir.AluOpType.min)
                    col = nb * 4 + q
                    nc.scalar.activation(
                        out=out_s[:, col:col + 1],
                        in_=mn[:],
                        func=mybir.ActivationFunctionType.Sqrt,
                        bias=xsq[:, col:col + 1],
                        scale=1.0,
                    )
            nc.sync.dma_start(out_d[:], out_s[:])

    nc.compile()
    return nc


def kernel(embedding: np.ndarray, memory_bank: np.ndarray) -> np.ndarray:
    emb = np.asarray(embedding, dtype=np.float32)
    bank = np.asarray(memory_bank, dtype=np.float32)
    assert emb.shape == (N, D) and bank.shape == (M, D)

    if "nc" not in _CACHE:
        _CACHE["nc"] = _build_kernel()
    nc = _CACHE["nc"]

    embT = np.ascontiguousarray(emb.T)
    x_sq = np.einsum("nd,nd->n", emb, emb, dtype=np.float64).astype(np.float32)
    xsq = np.ascontiguousarray(x_sq.reshape(N // 128, 128).T)

    in_maps = []
    for c in range(N_CORES):
        shard = bank[c * MS:(c + 1) * MS]
        bankT = np.ascontiguousarray((-2.0 * shard).T)
        m_sq = np.einsum("md,md->m", shard, shard, dtype=np.float64).astype(np.float32)
        msq = np.ascontiguousarray(m_sq.reshape(MS // 128, 128).T)
        in_maps.append({"embT": embT, "bankT": bankT, "msq": msq, "xsq": xsq})

    _CACHE["last_in_maps"] = in_maps
    try:
        res = run_bass_kernel_spmd(nc, in_maps, core_ids=list(range(N_CORES)))
    except Exception:
        # a previously-wedged NeuronCore reports unrecoverable once and then
        # recovers; one retry clears it
        import time
        time.sleep(2.0)
        res = run_bass_kernel_spmd(nc, in_maps, core_ids=list(range(N_CORES)))

    # gather: each core returns [128, N/128] local min distances; min over cores
    per_core = np.stack([res.results[c]["out"].T.reshape(N) for c in range(N_CORES)])
    return per_core.min(axis=0).reshape(N, 1).astype(np.float32)



# revision 3
# speedup vs baseline: 1.3007x; 1.3007x over previous
"""Trainium2 Bass kernel for nn_MemoryCore (retrieval KNN min-distance).

Problem: embedding [8192, 512], memory_bank [65536, 512] (fp32) ->
patch_scores [8192, 1] = min over the bank of euclidean distance.

Strategy (8 NeuronCores, SPMD):
  - Shard the memory bank (M axis) 8 ways; every core sees all queries.
  - fp8(e4m3) DoubleRow matmuls (contraction 256/instr, 2x PE throughput):
    psum[m, n] = (-2*bank_shard) @ emb.T. Bank tile stationary, reused
    across G=4 query blocks filling a 4-bank psum tile [128, 2048].
  - Evacuation alternates two paths to beat the DVE 1x-from-PSUM floor:
      * DVE: rm = min(psum + m_sq[m], rm)   (fused scalar_tensor_tensor)
      * ACT: tmp = bf16(psum + m_sq[m]); DVE: rm = min(tmp, rm) (2x bf16)
  - No device epilogue: ship rm [128, 8192] bf16; host does the
    cross-partition + cross-core min, adds ||x||^2, sqrt.
"""
import numpy as np
import ml_dtypes
import concourse.bacc as bacc
import concourse.mybir as mybir
import concourse.tile as tile
from concourse.bass_utils import run_bass_kernel_spmd

N_CORES = 8
N, M, D = 8192, 65536, 512
MS = M // N_CORES       # 8192 bank rows per core
MT = MS // 128          # 64 bank tiles (psum partition dim)
G = 4                   # query blocks (512 each) sharing one weight load
NG = N // (512 * G)     # 4 outer groups of 2048 queries
BIG = 1e30
DT = mybir.dt.float8e4  # e4m3 (TRN variant, max +-240): 2x PE with DoubleRow
DVE_EVERY = 4           # mt % DVE_EVERY == 0 -> direct DVE path, else ACT

_CACHE = {}


def _build_kernel():
    nc = bacc.Bacc("TRN2", target_bir_lowering=False, debug=False,
                   num_devices=N_CORES)

    embT_d = nc.dram_tensor("embT", [D, N], DT, kind="ExternalInput")
    bankT_d = nc.dram_tensor("bankT", [D, MS], DT, kind="ExternalInput")
    msq_d = nc.dram_tensor("msq", [128, MT], mybir.dt.float32, kind="ExternalInput")
    out_d = nc.dram_tensor("out", [128, N], mybir.dt.bfloat16, kind="ExternalOutput")

    GW = 512 * G  # 2048 queries per group

    with tile.TileContext(nc) as tc:
        with (
            tc.tile_pool(name="persist", bufs=1) as persist,
            tc.tile_pool(name="tmp", bufs=4) as tmpp,
            tc.tile_pool(name="psum", bufs=2, space="PSUM") as psum,
        ):
            msq = persist.tile([128, MT], mybir.dt.float32, tag="msq")
            nc.sync.dma_start(msq[:], msq_d[:])

            bank_t = persist.tile([128, 4, MS], DT, tag="bank")
            emb_t = persist.tile([128, 4, N], DT, tag="emb")
            for k in range(4):
                nc.sync.dma_start(bank_t[:, k, :],
                                  bankT_d[k * 128:(k + 1) * 128, :])
                nc.sync.dma_start(emb_t[:, k, :],
                                  embT_d[k * 128:(k + 1) * 128, :])

            rm_t = [persist.tile([128, GW], mybir.dt.bfloat16,
                                 name=f"rm{g}", tag=f"rm{g}")
                    for g in range(NG)]

            for g in range(NG):
                rm = rm_t[g]
                nc.vector.memset(rm[:], BIG)
                for mt in range(MT):
                    ps = psum.tile([128, GW], mybir.dt.float32, tag="ps")
                    for kp in range(2):
                        w = bank_t[:, kp * 2:(kp + 1) * 2,
                                   mt * 128:(mt + 1) * 128]
                        for j in range(G):
                            nb = g * G + j
                            nc.tensor.matmul(
                                ps[:, j * 512:(j + 1) * 512],
                                w,
                                emb_t[:, kp * 2:(kp + 1) * 2,
                                      nb * 512:(nb + 1) * 512],
                                start=(kp == 0),
                                stop=(kp == 1),
                                perf_mode=mybir.MatmulPerfMode.DoubleRow,
                            )
                    if mt % DVE_EVERY == 0:
                        # RM = min(psum + m_sq[m], RM)  (one DVE op from PSUM)
                        nc.vector.scalar_tensor_tensor(
                            out=rm[:],
                            in0=ps[:],
                            scalar=msq[:, mt:mt + 1],
                            in1=rm[:],
                            op0=mybir.AluOpType.add,
                            op1=mybir.AluOpType.min,
                        )
                    else:
                        # ACT evacuates (+ m_sq, downcast bf16); DVE mins at 2x
                        t = tmpp.tile([128, GW], mybir.dt.bfloat16, tag="t")
                        nc.scalar.activation(
                            out=t[:], in_=ps[:],
                            func=mybir.ActivationFunctionType.Identity,
                            bias=msq[:, mt:mt + 1],
                        )
                        nc.vector.scalar_tensor_tensor(
                            out=rm[:],
                            in0=t[:],
                            scalar=0.0,
                            in1=rm[:],
                            op0=mybir.AluOpType.add,
                            op1=mybir.AluOpType.min,
                        )
                nc.sync.dma_start(out_d[:, g * GW:(g + 1) * GW], rm[:])

    nc.compile()
    return nc


def kernel(embedding: np.ndarray, memory_bank: np.ndarray) -> np.ndarray:
    emb = np.asarray(embedding, dtype=np.float32)
    bank = np.asarray(memory_bank, dtype=np.float32)
    assert emb.shape == (N, D) and bank.shape == (M, D)

    if "nc" not in _CACHE:
        _CACHE["nc"] = _build_kernel()
    nc = _CACHE["nc"]

    embT8 = np.ascontiguousarray(emb.T).astype(ml_dtypes.float8_e4m3)
    x_sq = np.einsum("nd,nd->n", emb, emb, dtype=np.float64)  # [N]

    in_maps = []
    for c in range(N_CORES):
        shard = bank[c * MS:(c + 1) * MS]
        bankT8 = np.ascontiguousarray((-2.0 * shard).T).astype(
            ml_dtypes.float8_e4m3)
        m_sq = np.einsum("md,md->m", shard, shard,
                         dtype=np.float64).astype(np.float32)
        msq = np.ascontiguousarray(m_sq.reshape(MT, 128).T)
        in_maps.append({"embT": embT8, "bankT": bankT8, "msq": msq})

    _CACHE["last_in_maps"] = in_maps
    try:
        res = run_bass_kernel_spmd(nc, in_maps, core_ids=list(range(N_CORES)))
    except Exception:
        # a previously-wedged NeuronCore reports unrecoverable once and then
        # recovers; one retry clears it
        import time
        time.sleep(2.0)
        res = run_bass_kernel_spmd(nc, in_maps, core_ids=list(range(N_CORES)))

    # gather: each core returns [128, N] bf16 partial mins of (m_sq - 2 x.m);
    # min over partitions and cores, then + ||x||^2 and sqrt on host.
    per_core = np.stack([
        res.results[c]["out"].astype(np.float64).min(axis=0)
        for c in range(N_CORES)
    ])  # [8, N]
    tot = per_core.min(axis=0) + x_sq
    return np.sqrt(np.maximum(tot, 0.0)).astype(np.float32).reshape(N, 1)


# revision 6
# speedup vs baseline: 1.7104x; 1.3150x over previous
"""Trainium2 Bass kernel for nn_MemoryCore (retrieval KNN min-distance).

Problem: embedding [8192, 512], memory_bank [65536, 512] (fp32) ->
patch_scores [8192, 1] = min over the bank of euclidean distance.

Strategy (8 NeuronCores, SPMD):
  - Shard the memory bank (M axis) 8 ways; every core sees all queries.
  - fp8(e4m3) DoubleRow matmuls (contraction 256/instr, 2 fp8/cycle stream):
    psum[m, n] = (-2*bank_shard) @ emb.T. Bank tile stationary, reused
    across G=2 query blocks filling a 2-bank psum tile [128, 1024] (4 bufs).
  - PSUM evacuation is spread over three engine paths so no single engine
    exceeds the PE's ~440us of matmul streaming:
      D: DVE rm = min(psum + m_sq[m], rm)     (fused stt, 1x from PSUM)
      V: ACT tmp = bf16(psum + m_sq[m]); DVE  rm = min(tmp, rm)  (2x bf16)
      G: ACT tmp = bf16(psum + m_sq[m]); GPSIMD rm = min(tmp, rm)
  - No device epilogue: ship the bf16 running mins; host does the
    cross-partition + cross-core min, adds ||x||^2, sqrt.
"""
import numpy as np
import ml_dtypes
import concourse.bacc as bacc
import concourse.mybir as mybir
import concourse.tile as tile
from concourse.bass_utils import run_bass_kernel_spmd

N_CORES = 8
N, M, D = 8192, 65536, 512
MS = M // N_CORES       # 8192 bank rows per core
MT = MS // 128          # 64 bank tiles (psum partition dim)
G = 2                   # query blocks (512 each) sharing one weight load
GW = 512 * G            # 1024 queries per group
NGROUP = N // GW        # 8 groups
BIG = 1e30
DT = mybir.dt.float8e4  # e4m3 (TRN variant, max +-240): 2x PE with DoubleRow
# per-mt evacuation path: D=DVE direct stt, V=ACT evac + DVE bf16 min.
# 5/16 D + 11/16 V balances DVE (~400us) against ACT (~400us), both under
# the PE's ~440us of matmul streaming. (gpsimd can't run tensor_tensor.)
PATTERN = "DVVDVVDVVDVVVDVV"

_CACHE = {}


def _build_kernel():
    nc = bacc.Bacc("TRN2", target_bir_lowering=False, debug=False,
                   num_devices=N_CORES)

    embT_d = nc.dram_tensor("embT", [D, N], DT, kind="ExternalInput")
    bankT_d = nc.dram_tensor("bankT", [D, MS], DT, kind="ExternalInput")
    msq_d = nc.dram_tensor("msq", [128, MT], mybir.dt.float32, kind="ExternalInput")
    outv_d = nc.dram_tensor("outv", [128, N], mybir.dt.bfloat16,
                            kind="ExternalOutput")

    with tile.TileContext(nc) as tc:
        with (
            tc.tile_pool(name="persist", bufs=1) as persist,
            tc.tile_pool(name="tmp", bufs=6) as tmpp,
            tc.tile_pool(name="psum", bufs=4, space="PSUM") as psum,
        ):
            msq = persist.tile([128, MT], mybir.dt.float32, tag="msq")
            nc.sync.dma_start(msq[:], msq_d[:])

            bank_t = persist.tile([128, 4, MS], DT, tag="bank")
            emb_t = persist.tile([128, 4, N], DT, tag="emb")
            for k in range(4):
                nc.sync.dma_start(bank_t[:, k, :],
                                  bankT_d[k * 128:(k + 1) * 128, :])
                nc.sync.dma_start(emb_t[:, k, :],
                                  embT_d[k * 128:(k + 1) * 128, :])

            rm_v_t = [persist.tile([128, GW], mybir.dt.bfloat16,
                                   name=f"rmv{g}", tag=f"rmv{g}")
                      for g in range(NGROUP)]

            for gi in range(NGROUP):
                rm_v = rm_v_t[gi]
                nc.vector.memset(rm_v[:], BIG)
                for mt in range(MT):
                    ps = psum.tile([128, GW], mybir.dt.float32, tag="ps")
                    for kp in range(2):
                        w = bank_t[:, kp * 2:(kp + 1) * 2,
                                   mt * 128:(mt + 1) * 128]
                        for j in range(G):
                            nb = gi * G + j
                            nc.tensor.matmul(
                                ps[:, j * 512:(j + 1) * 512],
                                w,
                                emb_t[:, kp * 2:(kp + 1) * 2,
                                      nb * 512:(nb + 1) * 512],
                                start=(kp == 0),
                                stop=(kp == 1),
                                perf_mode=mybir.MatmulPerfMode.DoubleRow,
                            )
                    path = PATTERN[mt % len(PATTERN)]
                    if path == "D":
                        # rm = min(psum + m_sq[m], rm)  (one DVE op from PSUM)
                        nc.vector.scalar_tensor_tensor(
                            out=rm_v[:],
                            in0=ps[:],
                            scalar=msq[:, mt:mt + 1],
                            in1=rm_v[:],
                            op0=mybir.AluOpType.add,
                            op1=mybir.AluOpType.min,
                        )
                    else:
                        # ACT evacuates (+ m_sq, downcast bf16)
                        t = tmpp.tile([128, GW], mybir.dt.bfloat16, tag="t")
                        nc.scalar.activation(
                            out=t[:], in_=ps[:],
                            func=mybir.ActivationFunctionType.Identity,
                            bias=msq[:, mt:mt + 1],
                        )
                        nc.vector.tensor_tensor(
                            out=rm_v[:], in0=t[:], in1=rm_v[:],
                            op=mybir.AluOpType.min)
                nc.sync.dma_start(outv_d[:, gi * GW:(gi + 1) * GW], rm_v[:])

    nc.compile()
    return nc


def kernel(embedding: np.ndarray, memory_bank: np.ndarray) -> np.ndarray:
    emb = np.asarray(embedding, dtype=np.float32)
    bank = np.asarray(memory_bank, dtype=np.float32)
    assert emb.shape == (N, D) and bank.shape == (M, D)

    if "nc" not in _CACHE:
        _CACHE["nc"] = _build_kernel()
    nc = _CACHE["nc"]

    embT8 = np.ascontiguousarray(emb.T).astype(ml_dtypes.float8_e4m3)
    x_sq = np.einsum("nd,nd->n", emb, emb, dtype=np.float64)  # [N]

    in_maps = []
    for c in range(N_CORES):
        shard = bank[c * MS:(c + 1) * MS]
        bankT8 = np.ascontiguousarray((-2.0 * shard).T).astype(
            ml_dtypes.float8_e4m3)
        m_sq = np.einsum("md,md->m", shard, shard,
                         dtype=np.float64).astype(np.float32)
        msq = np.ascontiguousarray(m_sq.reshape(MT, 128).T)
        in_maps.append({"embT": embT8, "bankT": bankT8, "msq": msq})

    _CACHE["last_in_maps"] = in_maps
    try:
        res = run_bass_kernel_spmd(nc, in_maps, core_ids=list(range(N_CORES)))
    except Exception:
        # a previously-wedged NeuronCore reports unrecoverable once and then
        # recovers; one retry clears it
        import time
        time.sleep(2.0)
        res = run_bass_kernel_spmd(nc, in_maps, core_ids=list(range(N_CORES)))

    # gather: each core returns 2x [128, N] bf16 partial mins of
    # (m_sq - 2 x.m); min over paths, partitions and cores, then + ||x||^2
    # and sqrt on host.
    per_core = np.stack([
        res.results[c]["outv"].astype(np.float64).min(axis=0)
        for c in range(N_CORES)
    ])  # [8, N]
    tot = per_core.min(axis=0) + x_sq
    return np.sqrt(np.maximum(tot, 0.0)).astype(np.float32).reshape(N, 1)


# revision 8
# speedup vs baseline: 2.1125x; 1.2351x over previous
"""Trainium2 Bass kernel for nn_MemoryCore (retrieval KNN min-distance).

Problem: embedding [8192, 512], memory_bank [65536, 512] (fp32) ->
patch_scores [8192, 1] = min over the bank of euclidean distance.

Strategy (8 NeuronCores, SPMD):
  - Shard the memory bank (M axis) 8 ways; every core sees all queries.
  - fp8(e4m3) DoubleRow matmuls (contraction 256/instr, 2 fp8/cycle stream):
    psum[m, n] = (-2*bank_shard) @ emb.T. Bank tile stationary, reused
    across G=2 query blocks filling a 2-bank psum tile [128, 1024] (4 bufs).
  - PSUM evacuation is spread over three engine paths so no single engine
    exceeds the PE's ~440us of matmul streaming:
      D: DVE rm = min(psum + m_sq[m], rm)     (fused stt, 1x from PSUM)
      V: ACT tmp = bf16(psum + m_sq[m]); DVE  rm = min(tmp, rm)  (2x bf16)
      G: ACT tmp = bf16(psum + m_sq[m]); GPSIMD rm = min(tmp, rm)
  - No device epilogue: ship the bf16 running mins; host does the
    cross-partition + cross-core min, adds ||x||^2, sqrt.
"""
import numpy as np
import ml_dtypes
import concourse.bacc as bacc
import concourse.mybir as mybir
import concourse.tile as tile
from concourse.bass_utils import run_bass_kernel_spmd

N_CORES = 8
N, M, D = 8192, 65536, 512
MS = M // N_CORES       # 8192 bank rows per core
MT = MS // 128          # 64 bank tiles (psum partition dim)
G = 2                   # query blocks (512 each) sharing one weight load
GW = 512 * G            # 1024 queries per group
NGROUP = N // GW        # 8 groups
BIG = 1e30
DT = mybir.dt.float8e4  # e4m3 (TRN variant, max +-240): 2x PE with DoubleRow
# per-mt evacuation path: D=DVE direct stt, V=ACT evac + DVE bf16 min.
# Measured per-op costs (FD=1024): stt-from-PSUM 1541ns, ACTIVATE 1336ns,
# bf16 tensor_tensor 831ns. 1/4 D + 3/4 V balances DVE (~516us) against
# ACT (~513us), both under the PE's ~543us MM+LDW cadence.
PATTERN = "DVVVDVVVDVVVDVVV"

_CACHE = {}


def _build_kernel():
    nc = bacc.Bacc("TRN2", target_bir_lowering=False, debug=False,
                   num_devices=N_CORES)

    embT_d = nc.dram_tensor("embT", [D, N], DT, kind="ExternalInput")
    bankT_d = nc.dram_tensor("bankT", [D, MS], DT, kind="ExternalInput")
    msq_d = nc.dram_tensor("msq", [128, MT], mybir.dt.float32, kind="ExternalInput")
    outv_d = nc.dram_tensor("outv", [128, N], mybir.dt.bfloat16,
                            kind="ExternalOutput")

    with tile.TileContext(nc) as tc:
        with (
            tc.tile_pool(name="persist", bufs=1) as persist,
            tc.tile_pool(name="tmp", bufs=8) as tmpp,
            tc.tile_pool(name="psum", bufs=4, space="PSUM") as psum,
        ):
            msq = persist.tile([128, MT], mybir.dt.float32, tag="msq")
            nc.sync.dma_start(msq[:], msq_d[:])

            bank_t = persist.tile([128, 4, MS], DT, tag="bank")
            emb_t = persist.tile([128, 4, N], DT, tag="emb")
            for k in range(4):
                nc.sync.dma_start(bank_t[:, k, :],
                                  bankT_d[k * 128:(k + 1) * 128, :])
                nc.sync.dma_start(emb_t[:, k, :],
                                  embT_d[k * 128:(k + 1) * 128, :])

            rm_v_t = [persist.tile([128, GW], mybir.dt.bfloat16,
                                   name=f"rmv{g}", tag=f"rmv{g}")
                      for g in range(NGROUP)]

            for gi in range(NGROUP):
                rm_v = rm_v_t[gi]
                nc.vector.memset(rm_v[:], BIG)
                for mt in range(MT):
                    ps = psum.tile([128, GW], mybir.dt.float32, tag="ps")
                    for kp in range(2):
                        w = bank_t[:, kp * 2:(kp + 1) * 2,
                                   mt * 128:(mt + 1) * 128]
                        for j in range(G):
                            nb = gi * G + j
                            nc.tensor.matmul(
                                ps[:, j * 512:(j + 1) * 512],
                                w,
                                emb_t[:, kp * 2:(kp + 1) * 2,
                                      nb * 512:(nb + 1) * 512],
                                start=(kp == 0),
                                stop=(kp == 1),
                                perf_mode=mybir.MatmulPerfMode.DoubleRow,
                            )
                    path = PATTERN[mt % len(PATTERN)]
                    if path == "D":
                        # rm = min(psum + m_sq[m], rm)  (one DVE op from PSUM)
                        nc.vector.scalar_tensor_tensor(
                            out=rm_v[:],
                            in0=ps[:],
                            scalar=msq[:, mt:mt + 1],
                            in1=rm_v[:],
                            op0=mybir.AluOpType.add,
                            op1=mybir.AluOpType.min,
                        )
                    else:
                        # ACT evacuates (+ m_sq, downcast bf16)
                        t = tmpp.tile([128, GW], mybir.dt.bfloat16, tag="t")
                        nc.scalar.activation(
                            out=t[:], in_=ps[:],
                            func=mybir.ActivationFunctionType.Identity,
                            bias=msq[:, mt:mt + 1],
                        )
                        nc.vector.tensor_tensor(
                            out=rm_v[:], in0=t[:], in1=rm_v[:],
                            op=mybir.AluOpType.min)
                nc.sync.dma_start(outv_d[:, gi * GW:(gi + 1) * GW], rm_v[:])

    nc.compile()
    return nc


def kernel(embedding: np.ndarray, memory_bank: np.ndarray) -> np.ndarray:
    emb = np.asarray(embedding, dtype=np.float32)
    bank = np.asarray(memory_bank, dtype=np.float32)
    assert emb.shape == (N, D) and bank.shape == (M, D)

    if "nc" not in _CACHE:
        _CACHE["nc"] = _build_kernel()
    nc = _CACHE["nc"]

    embT8 = np.ascontiguousarray(emb.T).astype(ml_dtypes.float8_e4m3)
    x_sq = np.einsum("nd,nd->n", emb, emb, dtype=np.float64)  # [N]

    in_maps = []
    for c in range(N_CORES):
        shard = bank[c * MS:(c + 1) * MS]
        bankT8 = np.ascontiguousarray((-2.0 * shard).T).astype(
            ml_dtypes.float8_e4m3)
        m_sq = np.einsum("md,md->m", shard, shard,
                         dtype=np.float64).astype(np.float32)
        msq = np.ascontiguousarray(m_sq.reshape(MT, 128).T)
        in_maps.append({"embT": embT8, "bankT": bankT8, "msq": msq})

    _CACHE["last_in_maps"] = in_maps
    try:
        res = run_bass_kernel_spmd(nc, in_maps, core_ids=list(range(N_CORES)))
    except Exception:
        # a previously-wedged NeuronCore reports unrecoverable once and then
        # recovers; one retry clears it
        import time
        time.sleep(2.0)
        res = run_bass_kernel_spmd(nc, in_maps, core_ids=list(range(N_CORES)))

    # gather: each core returns 2x [128, N] bf16 partial mins of
    # (m_sq - 2 x.m); min over paths, partitions and cores, then + ||x||^2
    # and sqrt on host.
    per_core = np.stack([
        res.results[c]["outv"].astype(np.float64).min(axis=0)
        for c in range(N_CORES)
    ])  # [8, N]
    tot = per_core.min(axis=0) + x_sq
    return np.sqrt(np.maximum(tot, 0.0)).astype(np.float32).reshape(N, 1)


# revision 10
# speedup vs baseline: 2.1561x; 1.0207x over previous
"""Trainium2 Bass kernel for nn_MemoryCore (retrieval KNN min-distance).

Problem: embedding [8192, 512], memory_bank [65536, 512] (fp32) ->
patch_scores [8192, 1] = min over the bank of euclidean distance.

Strategy (8 NeuronCores, SPMD):
  - Shard the memory bank (M axis) 8 ways; every core sees all queries.
  - fp8(e4m3) DoubleRow matmuls (contraction 256/instr, 2 fp8/cycle stream):
    psum[m, n] = (-2*bank_shard) @ emb.T. Bank tile stationary, reused
    across G=2 query blocks filling a 2-bank psum tile [128, 1024] (4 bufs).
  - PSUM evacuation is spread over three engine paths so no single engine
    exceeds the PE's ~440us of matmul streaming:
      D: DVE rm = min(psum + m_sq[m], rm)     (fused stt, 1x from PSUM)
      V: ACT tmp = bf16(psum + m_sq[m]); DVE  rm = min(tmp, rm)  (2x bf16)
      G: ACT tmp = bf16(psum + m_sq[m]); GPSIMD rm = min(tmp, rm)
  - No device epilogue: ship the bf16 running mins; host does the
    cross-partition + cross-core min, adds ||x||^2, sqrt.
"""
import numpy as np
import ml_dtypes
import concourse.bacc as bacc
import concourse.mybir as mybir
import concourse.tile as tile
from concourse.bass_utils import run_bass_kernel_spmd

N_CORES = 8
N, M, D = 8192, 65536, 512
MS = M // N_CORES       # 8192 bank rows per core
MT = MS // 128          # 64 bank tiles (psum partition dim)
G = 2                   # query blocks (512 each) sharing one weight load
GW = 512 * G            # 1024 queries per group
NGROUP = N // GW        # 8 groups
BIG = 1e30
DT = mybir.dt.float8e4  # e4m3 (TRN variant, max +-240): 2x PE with DoubleRow
# per-mt evacuation path: D=DVE direct stt, V=ACT evac + DVE bf16 min.
# Measured per-op costs (FD=1024): stt-from-PSUM 1541ns, ACTIVATE 1336ns,
# bf16 tensor_tensor 831ns. 1/4 D + 3/4 V balances DVE (~516us) against
# ACT (~513us), both under the PE's ~543us MM+LDW cadence.
PATTERN = "DVVVDVVVDVVVDVVV"

_CACHE = {}


def _build_kernel():
    nc = bacc.Bacc("TRN2", target_bir_lowering=False, debug=False,
                   num_devices=N_CORES)

    embT_d = nc.dram_tensor("embT", [D, N], DT, kind="ExternalInput")
    bankT_d = nc.dram_tensor("bankT", [D, MS], DT, kind="ExternalInput")
    msq_d = nc.dram_tensor("msq", [128, MT], mybir.dt.float32, kind="ExternalInput")
    outv_d = nc.dram_tensor("outv", [128, N], mybir.dt.bfloat16,
                            kind="ExternalOutput")

    with tile.TileContext(nc) as tc:
        with (
            tc.tile_pool(name="persist", bufs=1) as persist,
            tc.tile_pool(name="tmp", bufs=10) as tmpp,
            tc.tile_pool(name="psum", bufs=4, space="PSUM") as psum,
        ):
            msq = persist.tile([128, MT], mybir.dt.float32, tag="msq")
            nc.sync.dma_start(msq[:], msq_d[:])

            bank_t = persist.tile([128, 4, MS], DT, tag="bank")
            emb_t = persist.tile([128, 4, N], DT, tag="emb")
            for k in range(4):
                nc.sync.dma_start(bank_t[:, k, :],
                                  bankT_d[k * 128:(k + 1) * 128, :])
                nc.sync.dma_start(emb_t[:, k, :],
                                  embT_d[k * 128:(k + 1) * 128, :])

            rm_v_t = [persist.tile([128, GW], mybir.dt.bfloat16,
                                   name=f"rmv{g}", tag=f"rmv{g}")
                      for g in range(NGROUP)]

            for gi in range(NGROUP):
                rm_v = rm_v_t[gi]
                nc.gpsimd.memset(rm_v[:], BIG)
                # bf16 mins are emitted lazily (LAG psum-units behind their
                # ACT) so a late ACT never blocks PSUM-critical stt ops at
                # the head of the DVE's strict FIFO.
                pending = []
                LAG = 2
                for mt in range(MT):
                    ps = psum.tile([128, GW], mybir.dt.float32, tag="ps")
                    for kp in range(2):
                        w = bank_t[:, kp * 2:(kp + 1) * 2,
                                   mt * 128:(mt + 1) * 128]
                        for j in range(G):
                            nb = gi * G + j
                            nc.tensor.matmul(
                                ps[:, j * 512:(j + 1) * 512],
                                w,
                                emb_t[:, kp * 2:(kp + 1) * 2,
                                      nb * 512:(nb + 1) * 512],
                                start=(kp == 0),
                                stop=(kp == 1),
                                perf_mode=mybir.MatmulPerfMode.DoubleRow,
                            )
                    path = PATTERN[mt % len(PATTERN)]
                    if path == "D":
                        # rm = min(psum + m_sq[m], rm)  (one DVE op from PSUM)
                        nc.vector.scalar_tensor_tensor(
                            out=rm_v[:],
                            in0=ps[:],
                            scalar=msq[:, mt:mt + 1],
                            in1=rm_v[:],
                            op0=mybir.AluOpType.add,
                            op1=mybir.AluOpType.min,
                        )
                    else:
                        # ACT evacuates (+ m_sq, downcast bf16)
                        t = tmpp.tile([128, GW], mybir.dt.bfloat16, tag="t")
                        nc.scalar.activation(
                            out=t[:], in_=ps[:],
                            func=mybir.ActivationFunctionType.Identity,
                            bias=msq[:, mt:mt + 1],
                        )
                        pending.append(t)
                    while len(pending) > LAG:
                        t = pending.pop(0)
                        nc.vector.tensor_tensor(
                            out=rm_v[:], in0=t[:], in1=rm_v[:],
                            op=mybir.AluOpType.min)
                for t in pending:
                    nc.vector.tensor_tensor(
                        out=rm_v[:], in0=t[:], in1=rm_v[:],
                        op=mybir.AluOpType.min)
                nc.sync.dma_start(outv_d[:, gi * GW:(gi + 1) * GW], rm_v[:])

    nc.compile()
    return nc


def kernel(embedding: np.ndarray, memory_bank: np.ndarray) -> np.ndarray:
    emb = np.asarray(embedding, dtype=np.float32)
    bank = np.asarray(memory_bank, dtype=np.float32)
    assert emb.shape == (N, D) and bank.shape == (M, D)

    if "nc" not in _CACHE:
        _CACHE["nc"] = _build_kernel()
    nc = _CACHE["nc"]

    embT8 = np.ascontiguousarray(emb.T).astype(ml_dtypes.float8_e4m3)
    x_sq = np.einsum("nd,nd->n", emb, emb, dtype=np.float64)  # [N]

    in_maps = []
    for c in range(N_CORES):
        shard = bank[c * MS:(c + 1) * MS]
        bankT8 = np.ascontiguousarray((-2.0 * shard).T).astype(
            ml_dtypes.float8_e4m3)
        m_sq = np.einsum("md,md->m", shard, shard,
                         dtype=np.float64).astype(np.float32)
        msq = np.ascontiguousarray(m_sq.reshape(MT, 128).T)
        in_maps.append({"embT": embT8, "bankT": bankT8, "msq": msq})

    _CACHE["last_in_maps"] = in_maps
    try:
        res = run_bass_kernel_spmd(nc, in_maps, core_ids=list(range(N_CORES)))
    except Exception:
        # a previously-wedged NeuronCore reports unrecoverable once and then
        # recovers; one retry clears it
        import time
        time.sleep(2.0)
        res = run_bass_kernel_spmd(nc, in_maps, core_ids=list(range(N_CORES)))

    # gather: each core returns 2x [128, N] bf16 partial mins of
    # (m_sq - 2 x.m); min over paths, partitions and cores, then + ||x||^2
    # and sqrt on host.
    per_core = np.stack([
        res.results[c]["outv"].astype(np.float64).min(axis=0)
        for c in range(N_CORES)
    ])  # [8, N]
    tot = per_core.min(axis=0) + x_sq
    return np.sqrt(np.maximum(tot, 0.0)).astype(np.float32).reshape(N, 1)


# revision 12
# speedup vs baseline: 2.2270x; 1.0329x over previous
"""Trainium2 Bass kernel for nn_MemoryCore (retrieval KNN min-distance).

Problem: embedding [8192, 512], memory_bank [65536, 512] (fp32) ->
patch_scores [8192, 1] = min over the bank of euclidean distance.

Strategy (8 NeuronCores, SPMD):
  - Shard the memory bank (M axis) 8 ways; every core sees all queries.
  - fp8(e4m3) DoubleRow matmuls (contraction 256/instr, 2 fp8/cycle stream):
    psum[m, n] = (-2*bank_shard) @ emb.T. Bank tile stationary, reused
    across G=2 query blocks filling a 2-bank psum tile [128, 1024] (4 bufs).
  - PSUM evacuation is spread over three engine paths so no single engine
    exceeds the PE's ~440us of matmul streaming:
      D: DVE rm = min(psum + m_sq[m], rm)     (fused stt, 1x from PSUM)
      V: ACT tmp = bf16(psum + m_sq[m]); DVE  rm = min(tmp, rm)  (2x bf16)
      G: ACT tmp = bf16(psum + m_sq[m]); GPSIMD rm = min(tmp, rm)
  - No device epilogue: ship the bf16 running mins; host does the
    cross-partition + cross-core min, adds ||x||^2, sqrt.
"""
import numpy as np
import ml_dtypes
import concourse.bacc as bacc
import concourse.mybir as mybir
import concourse.tile as tile
from concourse.bass_utils import run_bass_kernel_spmd

N_CORES = 8
N, M, D = 8192, 65536, 512
MS = M // N_CORES       # 8192 bank rows per core
MT = MS // 128          # 64 bank tiles (psum partition dim)
G = 2                   # query blocks (512 each) sharing one weight load
GW = 512 * G            # 1024 queries per group
NGROUP = N // GW        # 8 groups
BIG = 1e30
DT = mybir.dt.float8e4  # e4m3 (TRN variant, max +-240): 2x PE with DoubleRow
# per-mt evacuation path: D=DVE direct stt, V=ACT evac + DVE bf16 min.
# Measured per-op costs (FD=1024): stt-from-PSUM 1541ns, ACTIVATE 1336ns,
# bf16 tensor_tensor 831ns. 1/4 D + 3/4 V balances DVE (~516us) against
# ACT (~513us), both under the PE's ~543us MM+LDW cadence.
PATTERN = "DVVVDVVVDVVVDVVV"

_CACHE = {}


def _build_kernel():
    nc = bacc.Bacc("TRN2", target_bir_lowering=False, debug=False,
                   num_devices=N_CORES)

    embT_d = nc.dram_tensor("embT", [D, N], DT, kind="ExternalInput")
    bankT_d = nc.dram_tensor("bankT", [D, MS], DT, kind="ExternalInput")
    msq_d = nc.dram_tensor("msq", [128, MT], mybir.dt.float32, kind="ExternalInput")
    outv_d = nc.dram_tensor("outv", [128, N], mybir.dt.bfloat16,
                            kind="ExternalOutput")

    with tile.TileContext(nc) as tc:
        with (
            tc.tile_pool(name="persist", bufs=1) as persist,
            tc.tile_pool(name="tmp", bufs=10) as tmpp,
            tc.tile_pool(name="psum", bufs=4, space="PSUM") as psum,
        ):
            msq = persist.tile([128, MT], mybir.dt.float32, tag="msq")
            nc.sync.dma_start(msq[:], msq_d[:])

            bank_t = persist.tile([128, 4, MS], DT, tag="bank")
            emb_t = persist.tile([128, 4, N], DT, tag="emb")
            # all bank chunks first (every unit contracts over the full D),
            # then emb in per-group column slices so group 0's matmuls start
            # as soon as ~4.5MB (not 8MB) has landed.
            for k in range(4):
                nc.sync.dma_start(bank_t[:, k, :],
                                  bankT_d[k * 128:(k + 1) * 128, :])
            for gi in range(NGROUP):
                for k in range(4):
                    nc.sync.dma_start(
                        emb_t[:, k, gi * GW:(gi + 1) * GW],
                        embT_d[k * 128:(k + 1) * 128, gi * GW:(gi + 1) * GW])

            rm_v_t = [persist.tile([128, GW], mybir.dt.bfloat16,
                                   name=f"rmv{g}", tag=f"rmv{g}")
                      for g in range(NGROUP)]

            for gi in range(NGROUP):
                rm_v = rm_v_t[gi]
                nc.gpsimd.memset(rm_v[:], BIG)
                # bf16 mins are emitted lazily (LAG psum-units behind their
                # ACT) so a late ACT never blocks PSUM-critical stt ops at
                # the head of the DVE's strict FIFO.
                pending = []
                LAG = 4
                for mt in range(MT):
                    ps = psum.tile([128, GW], mybir.dt.float32, tag="ps")
                    for kp in range(2):
                        w = bank_t[:, kp * 2:(kp + 1) * 2,
                                   mt * 128:(mt + 1) * 128]
                        for j in range(G):
                            nb = gi * G + j
                            nc.tensor.matmul(
                                ps[:, j * 512:(j + 1) * 512],
                                w,
                                emb_t[:, kp * 2:(kp + 1) * 2,
                                      nb * 512:(nb + 1) * 512],
                                start=(kp == 0),
                                stop=(kp == 1),
                                perf_mode=mybir.MatmulPerfMode.DoubleRow,
                            )
                    path = PATTERN[mt % len(PATTERN)]
                    if path == "D":
                        # rm = min(psum + m_sq[m], rm)  (one DVE op from PSUM)
                        nc.vector.scalar_tensor_tensor(
                            out=rm_v[:],
                            in0=ps[:],
                            scalar=msq[:, mt:mt + 1],
                            in1=rm_v[:],
                            op0=mybir.AluOpType.add,
                            op1=mybir.AluOpType.min,
                        )
                    else:
                        # ACT evacuates (+ m_sq, downcast bf16)
                        t = tmpp.tile([128, GW], mybir.dt.bfloat16, tag="t")
                        nc.scalar.activation(
                            out=t[:], in_=ps[:],
                            func=mybir.ActivationFunctionType.Identity,
                            bias=msq[:, mt:mt + 1],
                        )
                        pending.append(t)
                    while len(pending) > LAG:
                        t = pending.pop(0)
                        nc.vector.tensor_tensor(
                            out=rm_v[:], in0=t[:], in1=rm_v[:],
                            op=mybir.AluOpType.min)
                for t in pending:
                    nc.vector.tensor_tensor(
                        out=rm_v[:], in0=t[:], in1=rm_v[:],
                        op=mybir.AluOpType.min)
                nc.sync.dma_start(outv_d[:, gi * GW:(gi + 1) * GW], rm_v[:])

    nc.compile()
    return nc


def kernel(embedding: np.ndarray, memory_bank: np.ndarray) -> np.ndarray:
    emb = np.asarray(embedding, dtype=np.float32)
    bank = np.asarray(memory_bank, dtype=np.float32)
    assert emb.shape == (N, D) and bank.shape == (M, D)

    if "nc" not in _CACHE:
        _CACHE["nc"] = _build_kernel()
    nc = _CACHE["nc"]

    embT8 = np.ascontiguousarray(emb.T).astype(ml_dtypes.float8_e4m3)
    x_sq = np.einsum("nd,nd->n", emb, emb, dtype=np.float64)  # [N]

    in_maps = []
    for c in range(N_CORES):
        shard = bank[c * MS:(c + 1) * MS]
        bankT8 = np.ascontiguousarray((-2.0 * shard).T).astype(
            ml_dtypes.float8_e4m3)
        m_sq = np.einsum("md,md->m", shard, shard,
                         dtype=np.float64).astype(np.float32)
        msq = np.ascontiguousarray(m_sq.reshape(MT, 128).T)
        in_maps.append({"embT": embT8, "bankT": bankT8, "msq": msq})

    _CACHE["last_in_maps"] = in_maps
    try:
        res = run_bass_kernel_spmd(nc, in_maps, core_ids=list(range(N_CORES)))
    except Exception:
        # a previously-wedged NeuronCore reports unrecoverable once and then
        # recovers; one retry clears it
        import time
        time.sleep(2.0)
        res = run_bass_kernel_spmd(nc, in_maps, core_ids=list(range(N_CORES)))

    # gather: each core returns 2x [128, N] bf16 partial mins of
    # (m_sq - 2 x.m); min over paths, partitions and cores, then + ||x||^2
    # and sqrt on host.
    per_core = np.stack([
        res.results[c]["outv"].astype(np.float64).min(axis=0)
        for c in range(N_CORES)
    ])  # [8, N]
    tot = per_core.min(axis=0) + x_sq
    return np.sqrt(np.maximum(tot, 0.0)).astype(np.float32).reshape(N, 1)
